# revision 13
# baseline (speedup 1.0000x reference)
"""Cross-image contrastive loss on 8 TRN2 NeuronCores — v3.

v2 -> v3: the exp+rowsum section was pipeline-stalled (each engine ~50%
idle) because ACT units (4 PSUM banks) and DVE units (4 banks) plus a
PE fill could not coexist in the 8-bank PSUM.  v3 partitions PSUM
6 banks / 2 banks:
  - ACT: two 1536-col groups (3 banks each) -> double-buffered; S1
    chunk-rows are consumed as 1536+1536+1024 activations.
  - DVE: one 1024-col group; pass 2 of the custom exp runs off-PSUM so
    the next fill overlaps it.
Also: S2 windows shrink to 768 cols, and the assignment is rebalanced
(ACT ~16.8us busy, DVE ~16.6us, PE ~14us at the observed 1.2GHz).

Algorithm (see v2 notes): host precomputes histograms/weights/diagonal;
rows and S2 columns label-sorted so diff_sum needs only a 768-col
window per 128-row chunk (CMASK fold in the K=84 augmented contraction
kills wrong-label and padding columns); device returns
sum_p (2/N) w_p log(Z_p + eps) per core; exp is split between the ACT
Exp activation (accum_out) and two custom DVE ops computing
exp(l/tau) ~= P(l/(512 tau))^512 by repeated squaring.
"""

import sys

import numpy as np

sys.path.insert(0, "/opt/trn_rl_repo")

import ml_dtypes

TAU = 0.07
EPS = 1e-4
L = 19
D = 64
N = 4096
NCORES = 8
P = N // NCORES  # 512 rows per core
KA = D + L + 1  # 84 augmented contraction for S2
CMASK = 4.25
W2 = 768  # S2 window columns per 128-row chunk
C0V = 1.0 / (TAU * 512.0)

_compiled = None
_EXPA8 = None
_SQ64SUM = None


def _ref_expa8(in0, in1, c0, c1, c2):
    x = np.asarray(in0, np.float32)
    c0 = np.float32(c0) if not isinstance(c0, np.ndarray) else c0
    c1 = np.float32(c1) if not isinstance(c1, np.ndarray) else c1
    v = (x * c0).astype(np.float32)
    p = ((1.0 + v) + (v * v) * c1).astype(np.float32)
    for _ in range(3):
        p = (p * p).astype(np.float32)
    return p


def _ref_sq64(in0, in1, c0, c1, c2):
    t = np.asarray(in0, np.float32)
    for _ in range(6):
        t = (t * t).astype(np.float32)
    return t, t.sum(axis=-1, keepdims=True)


def _register_dve_ops():
    global _EXPA8, _SQ64SUM
    if _EXPA8 is not None:
        return
    from concourse import dve_ops
    from concourse.dve_spec import AluOp, C1, One, Spec, Src0, lower, sq
    from concourse.dve_spec import C0 as C0leaf
    from concourse.dve_spec import _has_src1 as has_src1
    from concourse.dve_uop import DveOpSpec

    def mk(name, spec):
        for op in dve_ops.OPS:
            if op.name == name:
                return op
        row = dve_ops._CUSTOM_DVE_ROW_BASE + len(dve_ops.OPS)
        tmp = DveOpSpec(
            name=name, opcode=row, uops=lower(spec, ver="v3"), rd1_en=has_src1(spec)
        )
        op = dve_ops.DveOp(name, spec, subdim=False, uops_sha={"v3": tmp.sha("v3")})
        dve_ops.OPS.append(op)
        dve_ops._SUB_OPCODE_FOR_NAME[name] = row
        dve_ops.CUSTOM_DVE_SPECS[name] = spec
        return op

    v = Src0 * C0leaf
    body1 = (One + v) + sq(v) * C1  # 1 + v + v^2/2 with C1=0.5
    spec1 = Spec(body=sq(sq(sq(body1))), reference=_ref_expa8)
    spec2 = Spec(
        body=sq(sq(sq(sq(sq(sq(Src0)))))), accum=AluOp.ADD, reference=_ref_sq64
    )
    _EXPA8 = mk("ANT_EXPA8", spec1)
    _SQ64SUM = mk("ANT_SQ64SUM", spec2)


# ---- static unit schedule ---------------------------------------------------
# Per core: 4 chunks (128 rows each).  S1 = 4096 cols vs rhs1; S2 = one
# W2-col window vs rhs2.  Units: ("A"|"D", kind, chunk, col_off, width, jslot)
# jslot indexes the accumulator column: acc[:, 6*chunk + jslot].
# ACT: S1 of ch0, ch1 fully; ch2 cols 2048:4096; S2 w0,w1,w2.
# DVE: S1 ch2 cols 0:2048 (2x1024), ch3 fully (4x1024); S2 w3.
def _unit_schedule():
    A, Dv = [], []
    for b in (0, 1):
        A.append(("s1", b, 0, 1536, 0))
        A.append(("s1", b, 1536, 1536, 1))
        A.append(("s1", b, 3072, 1024, 2))
    A.append(("s1", 2, 2048, 1536, 2))
    A.append(("s1", 2, 3584, 512, 3))
    for b in (0, 1, 2):
        A.append(("s2", b, 0, W2, 5))
    for q in range(2):
        Dv.append(("s1", 2, q * 1024, 1024, q))
    for q in range(4):
        Dv.append(("s1", 3, q * 1024, 1024, q))
    Dv.append(("s2", 3, 0, W2, 5))
    return A, Dv


def _build():
    from concourse import bacc, mybir, tile

    _register_dve_ops()

    f32 = mybir.dt.float32
    bf16 = mybir.dt.bfloat16
    Exp = mybir.ActivationFunctionType.Exp
    Ln = mybir.ActivationFunctionType.Ln
    X = mybir.AxisListType.X
    add = mybir.AluOpType.add

    nc = bacc.Bacc("TRN2", target_bir_lowering=False, debug=False)

    lhs1_d = nc.dram_tensor("lhs1", (D, P), bf16, kind="ExternalInput")
    lhs2_d = nc.dram_tensor("lhs2", (KA, P), bf16, kind="ExternalInput")
    rhs1_d = nc.dram_tensor("rhs1", (D, N), bf16, kind="ExternalInput")
    rhs2_d = nc.dram_tensor("rhs2", (KA, 4 * W2), bf16, kind="ExternalInput")
    wz_d = nc.dram_tensor("wz", (128, 4), f32, kind="ExternalInput")
    out_d = nc.dram_tensor("out", (1, 1), f32, kind="ExternalOutput")

    with tile.TileContext(nc) as tc:
        with (
            tc.tile_pool(name="res", bufs=1) as res,
            tc.tile_pool(name="tsc", bufs=2) as tsc,
            tc.tile_pool(name="psA", bufs=2, space="PSUM") as psA,
            tc.tile_pool(name="psD", bufs=1, space="PSUM") as psD,
        ):
            lhs1_sb = res.tile([D, P], bf16, tag="lhs1")
            lhs2_sb = res.tile([KA, P], bf16, tag="lhs2")
            rhs1_sb = res.tile([D, N], bf16, tag="rhs1")
            rhs2_sb = res.tile([KA, 4 * W2], bf16, tag="rhs2")
            wz_sb = res.tile([128, 4], f32, tag="wz")
            accA = res.tile([128, 24], f32, tag="accA")
            accD = res.tile([128, 24], f32, tag="accD")
            dumpA = res.tile([128, 1536], bf16, tag="dumpA")
            junkD = res.tile([128, 1024], bf16, tag="junkD")
            zeros = res.tile([128, 1], f32, tag="zeros")
            ones = res.tile([128, 1], f32, tag="ones")

            # critical tensors first, spread across the two HWDGE queues
            nc.scalar.dma_start(lhs1_sb[:], lhs1_d[:])
            nc.sync.dma_start(rhs1_sb[:, 0:1024], rhs1_d[:, 0:1024])
            nc.scalar.dma_start(rhs1_sb[:, 1024:2048], rhs1_d[:, 1024:2048])
            nc.sync.dma_start(rhs1_sb[:, 2048:3072], rhs1_d[:, 2048:3072])
            nc.scalar.dma_start(rhs1_sb[:, 3072:4096], rhs1_d[:, 3072:4096])
            nc.sync.dma_start(lhs2_sb[:], lhs2_d[:])
            nc.sync.dma_start(rhs2_sb[:], rhs2_d[:])
            nc.sync.dma_start(wz_sb[:], wz_d[:])

            nc.scalar.add_instruction(
                mybir.InstLoadActFuncSet(
                    name=nc.get_next_instruction_name(),
                    act_func_set_id=6,
                    ins=[],
                    outs=[],
                )
            )

            nc.vector.memset(zeros[:], 0.0)
            nc.vector.memset(accA[:], 0.0)
            nc.gpsimd.memset(accD[:], 0.0)
            nc.gpsimd.memset(ones[:], 1.0)

            def fill(ps, kind, b, off, width):
                """Matmuls for unit (kind,b,off,width) into ps[:, 0:width]."""
                done = 0
                while done < width:
                    step = min(512, width - done)
                    if kind == "s1":
                        nc.tensor.matmul(
                            ps[:, done : done + step],
                            lhs1_sb[:, b * 128 : (b + 1) * 128],
                            rhs1_sb[:, off + done : off + done + step],
                            start=True,
                            stop=True,
                        )
                    else:
                        nc.tensor.matmul(
                            ps[:, done : done + step],
                            lhs2_sb[:, b * 128 : (b + 1) * 128],
                            rhs2_sb[:, b * W2 + done : b * W2 + done + step],
                            start=True,
                            stop=True,
                        )
                    done += step

            def act_unit(u):
                kind, b, off, width, j = u
                ps = psA.tile([128, 1536], f32, tag="mmA")
                fill(ps, kind, b, off, width)
                nc.scalar.activation(
                    dumpA[:, 0:width],
                    ps[:, 0:width],
                    Exp,
                    bias=zeros[:],
                    scale=1.0 / TAU,
                    accum_out=accA[:, 6 * b + j : 6 * b + j + 1],
                )

            def dve_unit(u):
                kind, b, off, width, j = u
                ps = psD.tile([128, 1024], f32, tag="mmD")
                fill(ps, kind, b, off, width)
                t = tsc.tile([128, 1024], f32, tag="t")
                nc.vector._custom_dve(
                    _EXPA8, out=t[:, 0:width], in0=ps[:, 0:width], s0=C0V, s1=0.5
                )
                nc.vector._custom_dve(
                    _SQ64SUM,
                    out=junkD[:, 0:width],
                    in0=t[:, 0:width],
                    accum_out=accD[:, 6 * b + j : 6 * b + j + 1],
                )

            A, Dv = _unit_schedule()
            # interleave: ACT first (scarcer engine), then alternate
            seq = []
            ai, di = 0, 0
            pattern = "ADADADADADADADAAAA"  # 11 A, 7 D
            for chix in pattern:
                if chix == "D" and di < len(Dv):
                    seq.append(("D", Dv[di]))
                    di += 1
                elif ai < len(A):
                    seq.append(("A", A[ai]))
                    ai += 1
            while di < len(Dv):
                seq.append(("D", Dv[di]))
                di += 1
            while ai < len(A):
                seq.append(("A", A[ai]))
                ai += 1
            for eng, u in seq:
                if eng == "D":
                    dve_unit(u)
                else:
                    act_unit(u)

            # ---- Z, logZ, partial ----
            zA = res.tile([128, 4], f32, tag="zA")
            zD = res.tile([128, 4], f32, tag="zD")
            nc.vector.tensor_reduce(
                zA[:], accA[:].rearrange("p (b j) -> p b j", j=6), axis=X, op=add
            )
            nc.vector.tensor_reduce(
                zD[:], accD[:].rearrange("p (b j) -> p b j", j=6), axis=X, op=add
            )
            zsum = res.tile([128, 4], f32, tag="zsum")
            nc.vector.tensor_add(zsum[:], zA[:], zD[:])
            nc.vector.tensor_scalar_add(zsum[:], zsum[:], EPS)
            logz = res.tile([128, 4], f32, tag="logz")
            nc.scalar.activation(logz[:], zsum[:], Ln, bias=zeros[:])

            vals = res.tile([128, 4], f32, tag="vals")
            nc.vector.tensor_mul(vals[:], logz[:], wz_sb[:])
            vred = res.tile([128, 1], f32, tag="vred")
            nc.vector.tensor_reduce(vred[:], vals[:], axis=X, op=add)
            fin = psD.tile([128, 1024], f32, tag="mmD")
            nc.tensor.matmul(fin[0:1, 0:1], ones[:], vred[:], start=True, stop=True)
            res_sb = res.tile([1, 1], f32, tag="res")
            nc.scalar.copy(res_sb[:], fin[0:1, 0:1])
            nc.sync.dma_start(out_d[:], res_sb[:])

    nc.compile()
    return nc


def _make_in_maps(features_i, features_ii, features_jj, i, ii, jj):
    """Host prep. Returns (in_maps, host_const)."""
    bf16 = ml_dtypes.bfloat16
    Fi = features_i.reshape(D, N).astype(np.float32)
    Fii = features_ii.reshape(D, N).astype(np.float32)
    Fjj = features_jj.reshape(D, N).astype(np.float32)
    lab = i.reshape(-1).astype(np.int64)
    ii_f = ii.reshape(-1).astype(np.int64)
    jj_f = jj.reshape(-1).astype(np.int64)

    cnt_ii = np.bincount(ii_f, minlength=L).astype(np.float32)
    cnt_jj = np.bincount(jj_f, minlength=L).astype(np.float32)
    wl = cnt_ii / (cnt_ii + cnt_jj + np.float32(EPS))
    w = wl[lab]
    diag = (Fi * (Fii + Fjj)).sum(axis=0) / np.float32(TAU)
    host_const = np.float32(-(w @ diag) / N)

    perm = np.argsort(lab, kind="stable")
    sFi = Fi[:, perm]
    slab = lab[perm]
    sw = w[perm]

    cperm = np.argsort(jj_f, kind="stable")
    sFjj = Fjj[:, cperm]
    sjj = jj_f[cperm]
    seg = np.searchsorted(sjj, np.arange(L + 1))

    G = np.zeros((KA, N + W2), np.float32)
    G[0:D, 0:N] = sFjj
    G[D + sjj, np.arange(N)] = CMASK
    G[D + L, :] = -CMASK

    woff = []
    for g in range(N // 128):
        la, lb = slab[g * 128], slab[g * 128 + 127]
        off, end = seg[la], seg[lb + 1]
        assert end - off <= W2, f"S2 window overflow: chunk {g}: {end - off}"
        woff.append(int(off))

    rhs1 = Fii.astype(bf16)

    in_maps = []
    for c in range(NCORES):
        sel = slice(c * P, (c + 1) * P)
        lhs1 = sFi[:, sel]
        sl = slab[sel]
        lhs2 = np.zeros((KA, P), np.float32)
        lhs2[0:D] = lhs1
        lhs2[D + sl, np.arange(P)] = 1.0
        lhs2[D + L] = 1.0
        rhs2 = np.concatenate(
            [G[:, woff[4 * c + b] : woff[4 * c + b] + W2] for b in range(4)], axis=1
        )
        wz = (np.float32(2.0) / N) * sw[sel].reshape(4, 128).T.copy()
        in_maps.append(
            {
                "lhs1": lhs1.astype(bf16),
                "lhs2": lhs2.astype(bf16),
                "rhs1": rhs1,
                "rhs2": rhs2.astype(bf16),
                "wz": wz.astype(np.float32),
            }
        )
    return in_maps, host_const


def kernel(features_i, features_ii, features_jj, i, ii, jj):
    global _compiled
    from concourse import bass_utils

    if _compiled is None:
        _compiled = _build()
    in_maps, host_const = _make_in_maps(
        features_i, features_ii, features_jj, i, ii, jj
    )
    results = bass_utils.run_bass_kernel_spmd(
        _compiled, in_maps, core_ids=list(range(NCORES))
    )
    total = np.float32(host_const)
    for r in results.results:
        total += np.float32(r["out"].reshape(-1)[0])
    return np.array(total, dtype=np.float32)


# revision 14
# speedup vs baseline: 1.0171x; 1.0171x over previous
"""Cross-image contrastive loss on 8 TRN2 NeuronCores — v3.

v2 -> v3: the exp+rowsum section was pipeline-stalled (each engine ~50%
idle) because ACT units (4 PSUM banks) and DVE units (4 banks) plus a
PE fill could not coexist in the 8-bank PSUM.  v3 partitions PSUM
6 banks / 2 banks:
  - ACT: two 1536-col groups (3 banks each) -> double-buffered; S1
    chunk-rows are consumed as 1536+1536+1024 activations.
  - DVE: one 1024-col group; pass 2 of the custom exp runs off-PSUM so
    the next fill overlaps it.
Also: S2 windows shrink to 768 cols, and the assignment is rebalanced
(ACT ~16.8us busy, DVE ~16.6us, PE ~14us at the observed 1.2GHz).

Algorithm (see v2 notes): host precomputes histograms/weights/diagonal;
rows and S2 columns label-sorted so diff_sum needs only a 768-col
window per 128-row chunk (CMASK fold in the K=84 augmented contraction
kills wrong-label and padding columns); device returns
sum_p (2/N) w_p log(Z_p + eps) per core; exp is split between the ACT
Exp activation (accum_out) and two custom DVE ops computing
exp(l/tau) ~= P(l/(512 tau))^512 by repeated squaring.
"""

import sys

import numpy as np

sys.path.insert(0, "/opt/trn_rl_repo")

import ml_dtypes

TAU = 0.07
EPS = 1e-4
L = 19
D = 64
N = 4096
NCORES = 8
P = N // NCORES  # 512 rows per core
KA = D + L + 1  # 84 augmented contraction for S2
CMASK = 4.25
W2 = 768  # S2 window columns per 128-row chunk
C0V = 1.0 / (TAU * 512.0)

_compiled = None
_EXPA8 = None
_SQ64SUM = None


def _ref_expa8(in0, in1, c0, c1, c2):
    x = np.asarray(in0, np.float32)
    c0 = np.float32(c0) if not isinstance(c0, np.ndarray) else c0
    c1 = np.float32(c1) if not isinstance(c1, np.ndarray) else c1
    v = (x * c0).astype(np.float32)
    p = ((1.0 + v) + (v * v) * c1).astype(np.float32)
    for _ in range(3):
        p = (p * p).astype(np.float32)
    return p


def _ref_sq64(in0, in1, c0, c1, c2):
    t = np.asarray(in0, np.float32)
    for _ in range(6):
        t = (t * t).astype(np.float32)
    return t, t.sum(axis=-1, keepdims=True)


def _register_dve_ops():
    global _EXPA8, _SQ64SUM
    if _EXPA8 is not None:
        return
    from concourse import dve_ops
    from concourse.dve_spec import AluOp, C1, One, Spec, Src0, lower, sq
    from concourse.dve_spec import C0 as C0leaf
    from concourse.dve_spec import _has_src1 as has_src1
    from concourse.dve_uop import DveOpSpec

    def mk(name, spec):
        for op in dve_ops.OPS:
            if op.name == name:
                return op
        row = dve_ops._CUSTOM_DVE_ROW_BASE + len(dve_ops.OPS)
        tmp = DveOpSpec(
            name=name, opcode=row, uops=lower(spec, ver="v3"), rd1_en=has_src1(spec)
        )
        op = dve_ops.DveOp(name, spec, subdim=False, uops_sha={"v3": tmp.sha("v3")})
        dve_ops.OPS.append(op)
        dve_ops._SUB_OPCODE_FOR_NAME[name] = row
        dve_ops.CUSTOM_DVE_SPECS[name] = spec
        return op

    v = Src0 * C0leaf
    body1 = (One + v) + sq(v) * C1  # 1 + v + v^2/2 with C1=0.5
    spec1 = Spec(body=sq(sq(sq(body1))), reference=_ref_expa8)
    spec2 = Spec(
        body=sq(sq(sq(sq(sq(sq(Src0)))))), accum=AluOp.ADD, reference=_ref_sq64
    )
    _EXPA8 = mk("ANT_EXPA8", spec1)
    _SQ64SUM = mk("ANT_SQ64SUM", spec2)


# ---- static unit schedule ---------------------------------------------------
# Per core: 4 chunks (128 rows each).  S1 = 4096 cols vs rhs1; S2 = one
# W2-col window vs rhs2.  Units: ("A"|"D", kind, chunk, col_off, width, jslot)
# jslot indexes the accumulator column: acc[:, 6*chunk + jslot].
# ACT: S1 of ch0, ch1 fully; ch2 cols 2048:4096; S2 w0,w1,w2.
# DVE: S1 ch2 cols 0:2048 (2x1024), ch3 fully (4x1024); S2 w3.
def _unit_schedule():
    A, Dv = [], []
    for b in (0, 1):
        A.append(("s1", b, 0, 1536, 0))
        A.append(("s1", b, 1536, 1536, 1))
        A.append(("s1", b, 3072, 1024, 2))
    A.append(("s1", 2, 2048, 1536, 2))
    A.append(("s1", 2, 3584, 512, 3))
    for b in (0, 1, 2):
        A.append(("s2", b, 0, W2, 5))
    for q in range(2):
        Dv.append(("s1", 2, q * 1024, 1024, q))
    for q in range(4):
        Dv.append(("s1", 3, q * 1024, 1024, q))
    Dv.append(("s2", 3, 0, W2, 5))
    return A, Dv


def _build():
    from concourse import bacc, mybir, tile

    _register_dve_ops()

    f32 = mybir.dt.float32
    bf16 = mybir.dt.bfloat16
    Exp = mybir.ActivationFunctionType.Exp
    Ln = mybir.ActivationFunctionType.Ln
    X = mybir.AxisListType.X
    add = mybir.AluOpType.add

    nc = bacc.Bacc("TRN2", target_bir_lowering=False, debug=False)

    lhs1_d = nc.dram_tensor("lhs1", (D, P), bf16, kind="ExternalInput")
    lhs2_d = nc.dram_tensor("lhs2", (KA, P), bf16, kind="ExternalInput")
    rhs1_d = nc.dram_tensor("rhs1", (D, N), bf16, kind="ExternalInput")
    rhs2_d = nc.dram_tensor("rhs2", (KA, 4 * W2), bf16, kind="ExternalInput")
    wz_d = nc.dram_tensor("wz", (128, 4), f32, kind="ExternalInput")
    out_d = nc.dram_tensor("out", (1, 1), f32, kind="ExternalOutput")

    with tile.TileContext(nc) as tc:
        with (
            tc.tile_pool(name="res", bufs=1) as res,
            tc.tile_pool(name="tsc", bufs=2) as tsc,
            tc.tile_pool(name="psA", bufs=2, space="PSUM") as psA,
            tc.tile_pool(name="psD", bufs=1, space="PSUM") as psD,
        ):
            lhs1_sb = res.tile([D, P], bf16, tag="lhs1")
            lhs2_sb = res.tile([KA, P], bf16, tag="lhs2")
            rhs1_sb = res.tile([D, N], bf16, tag="rhs1")
            rhs2_sb = res.tile([KA, 4 * W2], bf16, tag="rhs2")
            wz_sb = res.tile([128, 4], f32, tag="wz")
            accA = res.tile([128, 24], f32, tag="accA")
            accD = res.tile([128, 24], f32, tag="accD")
            dumpA = res.tile([128, 1536], bf16, tag="dumpA")
            junkD = res.tile([128, 1024], bf16, tag="junkD")
            zeros = res.tile([128, 1], f32, tag="zeros")
            ones = res.tile([128, 1], f32, tag="ones")

            # critical tensors first, spread across the two HWDGE queues
            nc.scalar.dma_start(lhs1_sb[:], lhs1_d[:])
            nc.sync.dma_start(rhs1_sb[:, 0:1024], rhs1_d[:, 0:1024])
            nc.scalar.dma_start(rhs1_sb[:, 1024:2048], rhs1_d[:, 1024:2048])
            nc.sync.dma_start(rhs1_sb[:, 2048:3072], rhs1_d[:, 2048:3072])
            nc.scalar.dma_start(rhs1_sb[:, 3072:4096], rhs1_d[:, 3072:4096])
            nc.sync.dma_start(lhs2_sb[:], lhs2_d[:])
            nc.sync.dma_start(rhs2_sb[:], rhs2_d[:])
            nc.sync.dma_start(wz_sb[:], wz_d[:])

            nc.scalar.add_instruction(
                mybir.InstLoadActFuncSet(
                    name=nc.get_next_instruction_name(),
                    act_func_set_id=6,
                    ins=[],
                    outs=[],
                )
            )

            nc.vector.memset(zeros[:], 0.0)
            nc.vector.memset(accA[:], 0.0)
            nc.gpsimd.memset(accD[:], 0.0)
            nc.gpsimd.memset(ones[:], 1.0)

            def fill(ps, kind, b, off, width):
                """Matmuls for unit (kind,b,off,width) into ps[:, 0:width]."""
                done = 0
                while done < width:
                    step = min(512, width - done)
                    if kind == "s1":
                        nc.tensor.matmul(
                            ps[:, done : done + step],
                            lhs1_sb[:, b * 128 : (b + 1) * 128],
                            rhs1_sb[:, off + done : off + done + step],
                            start=True,
                            stop=True,
                        )
                    else:
                        nc.tensor.matmul(
                            ps[:, done : done + step],
                            lhs2_sb[:, b * 128 : (b + 1) * 128],
                            rhs2_sb[:, b * W2 + done : b * W2 + done + step],
                            start=True,
                            stop=True,
                        )
                    done += step

            def act_unit(u):
                kind, b, off, width, j = u
                ps = psA.tile([128, 1536], f32, tag="mmA")
                fill(ps, kind, b, off, width)
                nc.scalar.activation(
                    dumpA[:, 0:width],
                    ps[:, 0:width],
                    Exp,
                    bias=zeros[:],
                    scale=1.0 / TAU,
                    accum_out=accA[:, 6 * b + j : 6 * b + j + 1],
                )

            def dve_unit(u):
                kind, b, off, width, j = u
                ps = psD.tile([128, 1024], f32, tag="mmD")
                fill(ps, kind, b, off, width)
                t = tsc.tile([128, 1024], f32, tag="t")
                nc.vector._custom_dve(
                    _EXPA8, out=t[:, 0:width], in0=ps[:, 0:width], s0=C0V, s1=0.5
                )
                nc.vector._custom_dve(
                    _SQ64SUM,
                    out=junkD[:, 0:width],
                    in0=t[:, 0:width],
                    accum_out=accD[:, 6 * b + j : 6 * b + j + 1],
                )

            A, Dv = _unit_schedule()
            # interleave: ACT first (scarcer engine), then alternate
            seq = []
            ai, di = 0, 0
            pattern = "DADADADADADAAAAAAA"  # 7 D, 11 A
            for chix in pattern:
                if chix == "D" and di < len(Dv):
                    seq.append(("D", Dv[di]))
                    di += 1
                elif ai < len(A):
                    seq.append(("A", A[ai]))
                    ai += 1
            while di < len(Dv):
                seq.append(("D", Dv[di]))
                di += 1
            while ai < len(A):
                seq.append(("A", A[ai]))
                ai += 1
            for eng, u in seq:
                if eng == "D":
                    dve_unit(u)
                else:
                    act_unit(u)

            # ---- Z, logZ, partial ----
            zA = res.tile([128, 4], f32, tag="zA")
            zD = res.tile([128, 4], f32, tag="zD")
            nc.vector.tensor_reduce(
                zA[:], accA[:].rearrange("p (b j) -> p b j", j=6), axis=X, op=add
            )
            nc.vector.tensor_reduce(
                zD[:], accD[:].rearrange("p (b j) -> p b j", j=6), axis=X, op=add
            )
            zsum = res.tile([128, 4], f32, tag="zsum")
            nc.vector.tensor_add(zsum[:], zA[:], zD[:])
            nc.vector.tensor_scalar_add(zsum[:], zsum[:], EPS)
            logz = res.tile([128, 4], f32, tag="logz")
            nc.scalar.activation(logz[:], zsum[:], Ln, bias=zeros[:])

            vals = res.tile([128, 4], f32, tag="vals")
            nc.vector.tensor_mul(vals[:], logz[:], wz_sb[:])
            vred = res.tile([128, 1], f32, tag="vred")
            nc.vector.tensor_reduce(vred[:], vals[:], axis=X, op=add)
            fin = psD.tile([128, 1024], f32, tag="mmD")
            nc.tensor.matmul(fin[0:1, 0:1], ones[:], vred[:], start=True, stop=True)
            res_sb = res.tile([1, 1], f32, tag="res")
            nc.scalar.copy(res_sb[:], fin[0:1, 0:1])
            nc.sync.dma_start(out_d[:], res_sb[:])

    nc.compile()
    return nc


def _make_in_maps(features_i, features_ii, features_jj, i, ii, jj):
    """Host prep. Returns (in_maps, host_const)."""
    bf16 = ml_dtypes.bfloat16
    Fi = features_i.reshape(D, N).astype(np.float32)
    Fii = features_ii.reshape(D, N).astype(np.float32)
    Fjj = features_jj.reshape(D, N).astype(np.float32)
    lab = i.reshape(-1).astype(np.int64)
    ii_f = ii.reshape(-1).astype(np.int64)
    jj_f = jj.reshape(-1).astype(np.int64)

    cnt_ii = np.bincount(ii_f, minlength=L).astype(np.float32)
    cnt_jj = np.bincount(jj_f, minlength=L).astype(np.float32)
    wl = cnt_ii / (cnt_ii + cnt_jj + np.float32(EPS))
    w = wl[lab]
    diag = (Fi * (Fii + Fjj)).sum(axis=0) / np.float32(TAU)
    host_const = np.float32(-(w @ diag) / N)

    perm = np.argsort(lab, kind="stable")
    sFi = Fi[:, perm]
    slab = lab[perm]
    sw = w[perm]

    cperm = np.argsort(jj_f, kind="stable")
    sFjj = Fjj[:, cperm]
    sjj = jj_f[cperm]
    seg = np.searchsorted(sjj, np.arange(L + 1))

    G = np.zeros((KA, N + W2), np.float32)
    G[0:D, 0:N] = sFjj
    G[D + sjj, np.arange(N)] = CMASK
    G[D + L, :] = -CMASK

    woff = []
    for g in range(N // 128):
        la, lb = slab[g * 128], slab[g * 128 + 127]
        off, end = seg[la], seg[lb + 1]
        assert end - off <= W2, f"S2 window overflow: chunk {g}: {end - off}"
        woff.append(int(off))

    rhs1 = Fii.astype(bf16)

    in_maps = []
    for c in range(NCORES):
        sel = slice(c * P, (c + 1) * P)
        lhs1 = sFi[:, sel]
        sl = slab[sel]
        lhs2 = np.zeros((KA, P), np.float32)
        lhs2[0:D] = lhs1
        lhs2[D + sl, np.arange(P)] = 1.0
        lhs2[D + L] = 1.0
        rhs2 = np.concatenate(
            [G[:, woff[4 * c + b] : woff[4 * c + b] + W2] for b in range(4)], axis=1
        )
        wz = (np.float32(2.0) / N) * sw[sel].reshape(4, 128).T.copy()
        in_maps.append(
            {
                "lhs1": lhs1.astype(bf16),
                "lhs2": lhs2.astype(bf16),
                "rhs1": rhs1,
                "rhs2": rhs2.astype(bf16),
                "wz": wz.astype(np.float32),
            }
        )
    return in_maps, host_const


def kernel(features_i, features_ii, features_jj, i, ii, jj):
    global _compiled
    from concourse import bass_utils

    if _compiled is None:
        _compiled = _build()
    in_maps, host_const = _make_in_maps(
        features_i, features_ii, features_jj, i, ii, jj
    )
    results = bass_utils.run_bass_kernel_spmd(
        _compiled, in_maps, core_ids=list(range(NCORES))
    )
    total = np.float32(host_const)
    for r in results.results:
        total += np.float32(r["out"].reshape(-1)[0])
    return np.array(total, dtype=np.float32)
